# revision 31
# baseline (speedup 1.0000x reference)
"""Single-head causal attention on 8 TRN2 NeuronCores.

Problem: x:[4,4096,1024] f32, Wq/Wk/Wv:[1024,64] f32.
  q,k,v = x@W*; scores = q@k.T/8 (causal); out = softmax(scores)@v.

Sharding: 2 cores per batch element (B=4 x 2 = 8 cores). Within a batch the
8 query tiles of 512 rows are dealt pairwise: core parity p owns absolute
tiles {2i+p}. Both cores run ONE SPMD program; the causal asymmetry between
even/odd tiles is pushed into per-core *data*: the key columns are staged in
per-core order [own_tile_i, other_tile_i]*4 and the causal masks are inputs.

Device algorithm (per core, all bf16 matmul operands, f32 PSUM accum):
  kvT[:,t]  = [Wk|Wv].T @ xT[:,t]          (stacked projection, kT + vT)
  qT        = Wq.T @ xT[:, own tiles]
  v_aug[kc] = transpose(vT chunk) ++ ones col   (PE transpose)
  scoresT   = kT_chunk.T @ qT_tile          ([128k x 512q] in PSUM)
  esT       = exp(0.125 * scoresT)          (ScalarE; no max-subtract needed:
                                             |scores| < ~4 so exp is safe)
  esT      *= mask (diagonal/padded chunks only)
  outT     += v_aug[kc].T @ esT             ([65 x 512]: 64 numerator rows +
                                             row 64 = softmax denominator)
  out_tile  = transpose(outT) ; out[:,0:64] * (1/out[:,64])
"""

import os
import numpy as np
import ml_dtypes

HEAD = 64
EMB = 1024
B = 4
T = 4096
QT = 512          # queries per logical tile (matmul moving dim)
NT = 4            # logical q tiles per core  (NT*QT = 2048 queries/core)
NKC = 32          # key chunks of 128 in the full sequence
P = 128
NCC = EMB // P    # contraction chunks for projections

_cache = {}
LAST_RESULT = None


def _build():
    import concourse.tile as tile
    import concourse.mybir as mybir
    from concourse import bacc
    from concourse.masks import make_identity

    bf16 = mybir.dt.bfloat16
    f32 = mybir.dt.float32
    Exp = mybir.ActivationFunctionType.Exp

    nc = bacc.Bacc(None)
    NS = T // QT
    # xkt is host-staged per 512-key span in exactly the SBUF tile layout
    # [span][c_part 128][c_chunk 8][q 512] so each span is one linear DMA
    xkt = nc.declare_dram_parameter("xkt", [NS, P, NCC, QT], bf16, isOutput=False)
    # weights and masks are host-staged in SBUF layout -> linear DMAs
    wq = nc.declare_dram_parameter("wq", [P, NCC, HEAD], bf16, isOutput=False)
    wkv = nc.declare_dram_parameter("wkv", [P, NCC, 2 * HEAD], bf16, isOutput=False)
    masks = nc.declare_dram_parameter("masks", [P, 8, QT], bf16, isOutput=False)
    out = nc.declare_dram_parameter("out", [NT * QT, HEAD], f32, isOutput=True)

    with tile.TileContext(nc) as tc:
        with (
            tc.tile_pool(name="const", bufs=1) as const,
            tc.tile_pool(name="xk", bufs=4) as xkp,
            tc.tile_pool(name="persist", bufs=1) as persist,
            tc.tile_pool(name="vt", bufs=3) as vtp,
            tc.tile_pool(name="es", bufs=4) as esp,
            tc.tile_pool(name="ot", bufs=2) as otp,
            tc.tile_pool(name="ob", bufs=4) as obp,
            tc.tile_pool(name="small", bufs=4) as smallp,
        ):
            # ---- constants / persistent SBUF ----
            wq_sb = const.tile([P, NCC, HEAD], bf16)
            nc.sync.dma_start(wq_sb[:], wq[:])
            wkv_sb = const.tile([P, NCC, 2 * HEAD], bf16)
            nc.sync.dma_start(wkv_sb[:], wkv[:])
            # masks are not needed until the attention phase; park them on the
            # slow software queue, off the critical first-span DMA path
            mask_sb = const.tile([P, 8, QT], bf16)
            nc.gpsimd.dma_start(mask_sb[:], masks[:])
            id_bf = const.tile([HEAD, HEAD], bf16)
            make_identity(nc, id_bf[:])
            id_f32 = const.tile([HEAD + 1, HEAD + 1], f32)
            make_identity(nc, id_f32[:])

            # kT/qT live duplicated in both partition halves so scores matmuls
            # can alternate PE row-groups (even kc -> rows 0-63, odd kc ->
            # rows 64-127), letting LDWEIGHTS overlap the neighboring matmul
            kt_sb = persist.tile([P, T], bf16, tag="kt")
            qt_sb = persist.tile([P, NT * QT], bf16, tag="qt")
            vaug_sb = persist.tile([P, NKC, HEAD + 1], bf16, tag="vaug")
            nc.vector.memset(vaug_sb[:, :, HEAD], 1.0)

            # ---- phase 1: projections (DMA-paced; PE stream skewed so the
            # v-transposes of span s run while span s+1's matmuls' inputs DMA) ----
            with (
                tc.tile_pool(name="ps_kv", bufs=2, space="PSUM") as ps_kv,
                tc.tile_pool(name="ps_q", bufs=2, space="PSUM") as ps_q,
                tc.tile_pool(name="ps_tr", bufs=2, space="PSUM") as ps_tr,
            ):
                vt_tiles = [None] * NS

                def emit_span(s):
                    xs = xkp.tile([P, NCC, QT], bf16, tag="xk")
                    if s == 0:
                        # split the critical first span across both HW queues
                        nc.sync.dma_start(xs[:, 0:NCC // 2, :], xkt[s, :, 0:NCC // 2, :])
                        nc.scalar.dma_start(xs[:, NCC // 2:NCC, :], xkt[s, :, NCC // 2:NCC, :])
                    else:
                        dma_eng = nc.sync if s % 2 == 0 else nc.scalar
                        dma_eng.dma_start(xs[:], xkt[s])
                    ps = ps_kv.tile([P, QT], mybir.dt.float32)
                    for j in range(NCC):
                        nc.tensor.matmul(
                            ps[:], lhsT=wkv_sb[:, j, :], rhs=xs[:, j, :],
                            start=(j == 0), stop=(j == NCC - 1),
                        )
                    sl = slice(s * QT, (s + 1) * QT)
                    nc.scalar.copy(kt_sb[0:HEAD, sl], ps[0:HEAD, :])
                    nc.vector.tensor_copy(kt_sb[HEAD:P, sl], ps[0:HEAD, :])
                    vt = vtp.tile([HEAD, QT], bf16, tag="vt")
                    nc.vector.tensor_copy(vt[:], ps[HEAD:P, :])
                    vt_tiles[s] = vt
                    if s % 2 == 0:
                        i = s // 2
                        psq = ps_q.tile([HEAD, QT], mybir.dt.float32)
                        for j in range(NCC):
                            nc.tensor.matmul(
                                psq[:], lhsT=wq_sb[:, j, :], rhs=xs[:, j, :],
                                start=(j == 0), stop=(j == NCC - 1),
                            )
                        qsl = slice(i * QT, (i + 1) * QT)
                        nc.scalar.copy(qt_sb[0:HEAD, qsl], psq[:])
                        nc.vector.tensor_copy(qt_sb[HEAD:P, qsl], psq[:])

                def emit_transposes(s):
                    vt = vt_tiles[s]
                    for bb in range(QT // P):
                        tp = ps_tr.tile([P, HEAD], bf16)
                        nc.tensor.transpose(tp[:], vt[:, bb * P:(bb + 1) * P], id_bf[:])
                        kc = s * (QT // P) + bb
                        nc.vector.tensor_copy(vaug_sb[:, kc, 0:HEAD], tp[:])

                for s in range(NS):
                    emit_span(s)
                    if s >= 1:
                        emit_transposes(s - 1)
                emit_transposes(NS - 1)

            # ---- phase 2: attention (PE stream skewed one chunk-pair: PV
            # matmuls of pair p are emitted after the scores matmuls of pair
            # p+1, so PE computes scores while ACT runs exp on the prior pair) ----
            GRP = 3  # key chunks per exp ACTIVATE (amortizes ACT fixed cost)
            with (
                tc.tile_pool(name="ps_sc", bufs=2, space="PSUM") as ps_sc,
                tc.tile_pool(name="ps_acc", bufs=1, space="PSUM") as ps_acc,
                tc.tile_pool(name="ps_fin", bufs=1, space="PSUM") as ps_fin,
            ):
                for i in range(NT):
                    nk = 8 * i + 8
                    acc = ps_acc.tile([HEAD + 1, QT], mybir.dt.float32)

                    def emit_scores(kc0, g):
                        sc = ps_sc.tile([P, GRP, QT], mybir.dt.float32, tag="sc")
                        for d in range(g):
                            kc = kc0 + d
                            h0 = (kc % 2) * HEAD  # alternate PE row-groups
                            nc.tensor.matmul(
                                sc[:, d, :],
                                lhsT=kt_sb[h0:h0 + HEAD, kc * P:(kc + 1) * P],
                                rhs=qt_sb[h0:h0 + HEAD, i * QT:(i + 1) * QT],
                                start=True, stop=True,
                            )
                        es = esp.tile([P, GRP, QT], bf16, tag="es")
                        nc.scalar.activation(
                            es[:, 0:g, :], sc[:, 0:g, :], Exp, scale=0.125)
                        for d in range(g):
                            kc = kc0 + d
                            if kc >= 8 * i:
                                nc.vector.tensor_mul(
                                    es[:, d, :], es[:, d, :],
                                    mask_sb[:, kc - 8 * i, :],
                                )
                        return es

                    def emit_pv(kc0, g, es):
                        for d in range(g):
                            kc = kc0 + d
                            nc.tensor.matmul(
                                acc[:],
                                lhsT=vaug_sb[:, kc, :],
                                rhs=es[:, d, :],
                                start=(kc == 0), stop=(kc == nk - 1),
                            )

                    from collections import deque
                    pending = deque()
                    kc0 = 0
                    while kc0 < nk:
                        g = min(GRP, nk - kc0)
                        es = emit_scores(kc0, g)
                        pending.append((kc0, g, es))
                        if len(pending) > 1:
                            emit_pv(*pending.popleft())
                        kc0 += g
                    while pending:
                        emit_pv(*pending.popleft())

                    ot = otp.tile([HEAD + 1, QT], mybir.dt.float32, tag="ot")
                    nc.vector.tensor_copy(ot[:], acc[:])
                    for sb in range(QT // P):
                        fp = ps_fin.tile([P, HEAD + 1], mybir.dt.float32)
                        nc.tensor.transpose(
                            fp[:], ot[:, sb * P:(sb + 1) * P], id_f32[:]
                        )
                        rc = smallp.tile([P, 1], mybir.dt.float32, tag="rc")
                        nc.vector.reciprocal(rc[:], fp[:, HEAD:HEAD + 1])
                        ob = obp.tile([P, HEAD], mybir.dt.float32, tag="ob")
                        nc.vector.tensor_scalar_mul(ob[:], fp[:, 0:HEAD], rc[:])
                        r0 = i * QT + sb * P
                        nc.sync.dma_start(out[r0:r0 + P, :], ob[:])
    nc.finalize()
    return nc


def _stage_inputs(x, Wq, Wk, Wv):
    bf = ml_dtypes.bfloat16

    def _w_stage(w):  # [1024, h] -> [128, 8, h] matching SBUF tiles
        w = np.asarray(w, dtype=np.float32).astype(bf)
        return np.ascontiguousarray(w.reshape(NCC, P, w.shape[1]).transpose(1, 0, 2))

    wq = _w_stage(Wq)
    wkv = _w_stage(np.concatenate([np.asarray(Wk), np.asarray(Wv)], axis=1))

    # causal mask slabs for the 4 diagonal key chunks of the own tile
    kk = np.arange(P)[:, None]
    qq = np.arange(QT)[None, :]
    tri = [(qq >= (P * j + kk)).astype(bf) for j in range(4)]
    zeros = np.zeros((P, QT), dtype=bf)
    ones = np.ones((P, QT), dtype=bf)
    # device layout [128, 8, 512]
    mask_even = np.ascontiguousarray(
        np.stack(tri + [zeros] * 4).transpose(1, 0, 2))  # own first: future -> 0
    mask_odd = np.ascontiguousarray(
        np.stack(tri + [ones] * 4).transpose(1, 0, 2))   # own second: past -> 1

    in_maps = []
    for b in range(B):
        xbt = np.ascontiguousarray(x[b].T, dtype=np.float32).astype(bf)
        for p in range(2):
            cols = []
            for i in range(NT):
                own = 2 * i + p
                oth = 2 * i + 1 - p
                cols.append(xbt[:, own * QT:(own + 1) * QT])
                cols.append(xbt[:, oth * QT:(oth + 1) * QT])
            staged = np.concatenate(cols, axis=1)  # [1024, 4096]
            # device layout: [span][c_part 128][c_chunk 8][q 512]
            staged = np.ascontiguousarray(
                staged.reshape(NCC, P, T // QT, QT).transpose(2, 1, 0, 3)
            )
            in_maps.append({
                "xkt": staged,
                "wq": wq,
                "wkv": wkv,
                "masks": mask_even if p == 0 else mask_odd,
            })
    return in_maps


def kernel(x, Wq, Wk, Wv):
    global LAST_RESULT
    from concourse.bass_utils import run_bass_kernel_spmd

    x = np.asarray(x)
    if "nc" not in _cache:
        _cache["nc"] = _build()
    nc = _cache["nc"]

    in_maps = _stage_inputs(x, Wq, Wk, Wv)
    trace = bool(int(os.environ.get("ATTN_TRACE", "0")))
    res = run_bass_kernel_spmd(nc, in_maps, core_ids=list(range(8)), trace=trace)
    LAST_RESULT = res

    out = np.empty((B, T, HEAD), dtype=np.float32)
    for b in range(B):
        for p in range(2):
            o = res.results[2 * b + p]["out"]
            for i in range(NT):
                a0 = (2 * i + p) * QT
                out[b, a0:a0 + QT] = o[i * QT:(i + 1) * QT]
    return out


# revision 33
# speedup vs baseline: 1.0266x; 1.0266x over previous
"""Single-head causal attention on 8 TRN2 NeuronCores.

Problem: x:[4,4096,1024] f32, Wq/Wk/Wv:[1024,64] f32.
  q,k,v = x@W*; scores = q@k.T/8 (causal); out = softmax(scores)@v.

Sharding: 2 cores per batch element (B=4 x 2 = 8 cores). Within a batch the
8 query tiles of 512 rows are dealt pairwise: core parity p owns absolute
tiles {2i+p}. Both cores run ONE SPMD program; the causal asymmetry between
even/odd tiles is pushed into per-core *data*: the key columns are staged in
per-core order [own_tile_i, other_tile_i]*4 and the causal masks are inputs.

Device algorithm (per core, all bf16 matmul operands, f32 PSUM accum):
  kvT[:,t]  = [Wk|Wv].T @ xT[:,t]          (stacked projection, kT + vT)
  qT        = Wq.T @ xT[:, own tiles]
  v_aug[kc] = transpose(vT chunk) ++ ones col   (PE transpose)
  scoresT   = kT_chunk.T @ qT_tile          ([128k x 512q] in PSUM)
  esT       = exp(0.125 * scoresT)          (ScalarE; no max-subtract needed:
                                             |scores| < ~4 so exp is safe)
  esT      *= mask (diagonal/padded chunks only)
  outT     += v_aug[kc].T @ esT             ([65 x 512]: 64 numerator rows +
                                             row 64 = softmax denominator)
  out_tile  = transpose(outT) ; out[:,0:64] * (1/out[:,64])
"""

import os
import numpy as np
import ml_dtypes

HEAD = 64
EMB = 1024
B = 4
T = 4096
QT = 512          # queries per logical tile (matmul moving dim)
NT = 4            # logical q tiles per core  (NT*QT = 2048 queries/core)
NKC = 32          # key chunks of 128 in the full sequence
P = 128
NCC = EMB // P    # contraction chunks for projections

_cache = {}
LAST_RESULT = None


def _build():
    import concourse.tile as tile
    import concourse.mybir as mybir
    from concourse import bacc
    from concourse.masks import make_identity

    bf16 = mybir.dt.bfloat16
    f32 = mybir.dt.float32
    Exp = mybir.ActivationFunctionType.Exp

    nc = bacc.Bacc(None)
    NS = T // QT
    # xkt is host-staged per 512-key span in exactly the SBUF tile layout
    # [span][c_part 128][c_chunk 8][q 512] so each span is one linear DMA
    xkt = nc.declare_dram_parameter("xkt", [NS, P, NCC, QT], bf16, isOutput=False)
    # weights and masks are host-staged in SBUF layout -> linear DMAs
    wq = nc.declare_dram_parameter("wq", [P, NCC, HEAD], bf16, isOutput=False)
    wkv = nc.declare_dram_parameter("wkv", [P, NCC, 2 * HEAD], bf16, isOutput=False)
    masks = nc.declare_dram_parameter("masks", [P, 8, QT], bf16, isOutput=False)
    out = nc.declare_dram_parameter("out", [NT * QT, HEAD], f32, isOutput=True)

    with tile.TileContext(nc) as tc:
        with (
            tc.tile_pool(name="const", bufs=1) as const,
            tc.tile_pool(name="xk", bufs=4) as xkp,
            tc.tile_pool(name="persist", bufs=1) as persist,
            tc.tile_pool(name="vt", bufs=3) as vtp,
            tc.tile_pool(name="es", bufs=4) as esp,
            tc.tile_pool(name="ot", bufs=2) as otp,
            tc.tile_pool(name="ob", bufs=4) as obp,
            tc.tile_pool(name="small", bufs=4) as smallp,
        ):
            # ---- constants / persistent SBUF ----
            wq_sb = const.tile([P, NCC, HEAD], bf16)
            nc.sync.dma_start(wq_sb[:], wq[:])
            wkv_sb = const.tile([P, NCC, 2 * HEAD], bf16)
            nc.sync.dma_start(wkv_sb[:], wkv[:])
            # masks are not needed until the attention phase; park them on the
            # slow software queue, off the critical first-span DMA path
            mask_sb = const.tile([P, 8, QT], bf16)
            nc.gpsimd.dma_start(mask_sb[:], masks[:])
            id_bf = const.tile([HEAD, HEAD], bf16)
            make_identity(nc, id_bf[:])
            id_f32 = const.tile([HEAD + 1, HEAD + 1], f32)
            make_identity(nc, id_f32[:])

            # kT/qT live duplicated in both partition halves so scores matmuls
            # can alternate PE row-groups (even kc -> rows 0-63, odd kc ->
            # rows 64-127), letting LDWEIGHTS overlap the neighboring matmul
            kt_sb = persist.tile([P, T], bf16, tag="kt")
            qt_sb = persist.tile([P, NT * QT], bf16, tag="qt")
            vaug_sb = persist.tile([P, NKC, HEAD + 1], bf16, tag="vaug")
            nc.vector.memset(vaug_sb[:, :, HEAD], 1.0)

            # ---- phase 1: projections (DMA-paced; PE stream skewed so the
            # v-transposes of span s run while span s+1's matmuls' inputs DMA) ----
            with (
                tc.tile_pool(name="ps_kv", bufs=2, space="PSUM") as ps_kv,
                tc.tile_pool(name="ps_q", bufs=2, space="PSUM") as ps_q,
                tc.tile_pool(name="ps_tr", bufs=2, space="PSUM") as ps_tr,
            ):
                vt_tiles = [None] * NS

                def emit_span(s):
                    xs = xkp.tile([P, NCC, QT], bf16, tag="xk")
                    if s == 0:
                        # split the critical first span across both HW queues
                        nc.sync.dma_start(xs[:, 0:NCC // 2, :], xkt[s, :, 0:NCC // 2, :])
                        nc.scalar.dma_start(xs[:, NCC // 2:NCC, :], xkt[s, :, NCC // 2:NCC, :])
                    else:
                        dma_eng = nc.sync if s % 2 == 0 else nc.scalar
                        dma_eng.dma_start(xs[:], xkt[s])
                    ps = ps_kv.tile([P, QT], mybir.dt.float32)
                    for j in range(NCC):
                        nc.tensor.matmul(
                            ps[:], lhsT=wkv_sb[:, j, :], rhs=xs[:, j, :],
                            start=(j == 0), stop=(j == NCC - 1),
                        )
                    sl = slice(s * QT, (s + 1) * QT)
                    nc.scalar.copy(kt_sb[0:HEAD, sl], ps[0:HEAD, :])
                    nc.vector.tensor_copy(kt_sb[HEAD:P, sl], ps[0:HEAD, :])
                    vt = vtp.tile([HEAD, QT], bf16, tag="vt")
                    nc.vector.tensor_copy(vt[:], ps[HEAD:P, :])
                    vt_tiles[s] = vt
                    if s % 2 == 0:
                        i = s // 2
                        psq = ps_q.tile([HEAD, QT], mybir.dt.float32)
                        for j in range(NCC):
                            nc.tensor.matmul(
                                psq[:], lhsT=wq_sb[:, j, :], rhs=xs[:, j, :],
                                start=(j == 0), stop=(j == NCC - 1),
                            )
                        qsl = slice(i * QT, (i + 1) * QT)
                        nc.scalar.copy(qt_sb[0:HEAD, qsl], psq[:])
                        nc.vector.tensor_copy(qt_sb[HEAD:P, qsl], psq[:])

                def emit_transposes(s):
                    vt = vt_tiles[s]
                    for bb in range(QT // P):
                        tp = ps_tr.tile([P, HEAD], bf16)
                        nc.tensor.transpose(tp[:], vt[:, bb * P:(bb + 1) * P], id_bf[:])
                        kc = s * (QT // P) + bb
                        nc.vector.tensor_copy(vaug_sb[:, kc, 0:HEAD], tp[:])

                for s in range(NS):
                    emit_span(s)
                    if s >= 1:
                        emit_transposes(s - 1)
                emit_transposes(NS - 1)

            # ---- phase 2: attention (PE stream skewed one chunk-pair: PV
            # matmuls of pair p are emitted after the scores matmuls of pair
            # p+1, so PE computes scores while ACT runs exp on the prior pair) ----
            GRP = 2  # key chunks per exp ACTIVATE (amortizes ACT fixed cost)
            with (
                tc.tile_pool(name="ps_sc", bufs=3, space="PSUM") as ps_sc,
                tc.tile_pool(name="ps_acc", bufs=1, space="PSUM") as ps_acc,
                tc.tile_pool(name="ps_fin", bufs=1, space="PSUM") as ps_fin,
            ):
                for i in range(NT):
                    nk = 8 * i + 8
                    acc = ps_acc.tile([HEAD + 1, QT], mybir.dt.float32)

                    def emit_scores(kc0, g):
                        sc = ps_sc.tile([P, GRP, QT], mybir.dt.float32, tag="sc")
                        for d in range(g):
                            kc = kc0 + d
                            h0 = (kc % 2) * HEAD  # alternate PE row-groups
                            nc.tensor.matmul(
                                sc[:, d, :],
                                lhsT=kt_sb[h0:h0 + HEAD, kc * P:(kc + 1) * P],
                                rhs=qt_sb[h0:h0 + HEAD, i * QT:(i + 1) * QT],
                                start=True, stop=True,
                            )
                        es = esp.tile([P, GRP, QT], bf16, tag="es")
                        nc.scalar.activation(
                            es[:, 0:g, :], sc[:, 0:g, :], Exp, scale=0.125)
                        for d in range(g):
                            kc = kc0 + d
                            if kc >= 8 * i:
                                nc.vector.tensor_mul(
                                    es[:, d, :], es[:, d, :],
                                    mask_sb[:, kc - 8 * i, :],
                                )
                        return es

                    def emit_pv(kc0, g, es):
                        for d in range(g):
                            kc = kc0 + d
                            nc.tensor.matmul(
                                acc[:],
                                lhsT=vaug_sb[:, kc, :],
                                rhs=es[:, d, :],
                                start=(kc == 0), stop=(kc == nk - 1),
                            )

                    from collections import deque
                    pending = deque()
                    kc0 = 0
                    while kc0 < nk:
                        g = min(GRP, nk - kc0)
                        es = emit_scores(kc0, g)
                        pending.append((kc0, g, es))
                        if len(pending) > 2:
                            emit_pv(*pending.popleft())
                        kc0 += g
                    while pending:
                        emit_pv(*pending.popleft())

                    ot = otp.tile([HEAD + 1, QT], mybir.dt.float32, tag="ot")
                    nc.vector.tensor_copy(ot[:], acc[:])
                    for sb in range(QT // P):
                        fp = ps_fin.tile([P, HEAD + 1], mybir.dt.float32)
                        nc.tensor.transpose(
                            fp[:], ot[:, sb * P:(sb + 1) * P], id_f32[:]
                        )
                        rc = smallp.tile([P, 1], mybir.dt.float32, tag="rc")
                        nc.vector.reciprocal(rc[:], fp[:, HEAD:HEAD + 1])
                        ob = obp.tile([P, HEAD], mybir.dt.float32, tag="ob")
                        nc.vector.tensor_scalar_mul(ob[:], fp[:, 0:HEAD], rc[:])
                        r0 = i * QT + sb * P
                        nc.sync.dma_start(out[r0:r0 + P, :], ob[:])
    nc.finalize()
    return nc


def _stage_inputs(x, Wq, Wk, Wv):
    bf = ml_dtypes.bfloat16

    def _w_stage(w):  # [1024, h] -> [128, 8, h] matching SBUF tiles
        w = np.asarray(w, dtype=np.float32).astype(bf)
        return np.ascontiguousarray(w.reshape(NCC, P, w.shape[1]).transpose(1, 0, 2))

    wq = _w_stage(Wq)
    wkv = _w_stage(np.concatenate([np.asarray(Wk), np.asarray(Wv)], axis=1))

    # causal mask slabs for the 4 diagonal key chunks of the own tile
    kk = np.arange(P)[:, None]
    qq = np.arange(QT)[None, :]
    tri = [(qq >= (P * j + kk)).astype(bf) for j in range(4)]
    zeros = np.zeros((P, QT), dtype=bf)
    ones = np.ones((P, QT), dtype=bf)
    # device layout [128, 8, 512]
    mask_even = np.ascontiguousarray(
        np.stack(tri + [zeros] * 4).transpose(1, 0, 2))  # own first: future -> 0
    mask_odd = np.ascontiguousarray(
        np.stack(tri + [ones] * 4).transpose(1, 0, 2))   # own second: past -> 1

    in_maps = []
    for b in range(B):
        xbt = np.ascontiguousarray(x[b].T, dtype=np.float32).astype(bf)
        for p in range(2):
            cols = []
            for i in range(NT):
                own = 2 * i + p
                oth = 2 * i + 1 - p
                cols.append(xbt[:, own * QT:(own + 1) * QT])
                cols.append(xbt[:, oth * QT:(oth + 1) * QT])
            staged = np.concatenate(cols, axis=1)  # [1024, 4096]
            # device layout: [span][c_part 128][c_chunk 8][q 512]
            staged = np.ascontiguousarray(
                staged.reshape(NCC, P, T // QT, QT).transpose(2, 1, 0, 3)
            )
            in_maps.append({
                "xkt": staged,
                "wq": wq,
                "wkv": wkv,
                "masks": mask_even if p == 0 else mask_odd,
            })
    return in_maps


def kernel(x, Wq, Wk, Wv):
    global LAST_RESULT
    from concourse.bass_utils import run_bass_kernel_spmd

    x = np.asarray(x)
    if "nc" not in _cache:
        _cache["nc"] = _build()
    nc = _cache["nc"]

    in_maps = _stage_inputs(x, Wq, Wk, Wv)
    trace = bool(int(os.environ.get("ATTN_TRACE", "0")))
    res = run_bass_kernel_spmd(nc, in_maps, core_ids=list(range(8)), trace=trace)
    LAST_RESULT = res

    out = np.empty((B, T, HEAD), dtype=np.float32)
    for b in range(B):
        for p in range(2):
            o = res.results[2 * b + p]["out"]
            for i in range(NT):
                a0 = (2 * i + p) * QT
                out[b, a0:a0 + QT] = o[i * QT:(i + 1) * QT]
    return out


# revision 39
# speedup vs baseline: 1.0845x; 1.0564x over previous
"""Single-head causal attention on 8 TRN2 NeuronCores.

Problem: x:[4,4096,1024] f32, Wq/Wk/Wv:[1024,64] f32.
  q,k,v = x@W*; scores = q@k.T/8 (causal); out = softmax(scores)@v.

Sharding: 2 cores per batch element (B=4 x 2 = 8 cores). Within a batch the
8 query tiles of 512 rows are dealt pairwise: core parity p owns absolute
tiles {2i+p}. Both cores run ONE SPMD program; the causal asymmetry between
even/odd tiles is pushed into per-core *data*: the key columns are staged in
per-core order [own_tile_i, other_tile_i]*4 and the causal masks are inputs.

Device algorithm (per core, all bf16 matmul operands, f32 PSUM accum):
  kvT[:,t]  = [Wk|Wv].T @ xT[:,t]          (stacked projection, kT + vT)
  qT        = Wq.T @ xT[:, own tiles]
  v_aug[kc] = transpose(vT chunk) ++ ones col   (PE transpose)
  scoresT   = kT_chunk.T @ qT_tile          ([128k x 512q] in PSUM)
  esT       = exp(0.125 * scoresT)          (ScalarE; no max-subtract needed:
                                             |scores| < ~4 so exp is safe)
  esT      *= mask (diagonal/padded chunks only)
  outT     += v_aug[kc].T @ esT             ([65 x 512]: 64 numerator rows +
                                             row 64 = softmax denominator)
  out_tile  = transpose(outT) ; out[:,0:64] * (1/out[:,64])
"""

import os
import numpy as np
import ml_dtypes

HEAD = 64
EMB = 1024
B = 4
T = 4096
QT = 512          # queries per logical tile (matmul moving dim)
NT = 4            # logical q tiles per core  (NT*QT = 2048 queries/core)
NKC = 32          # key chunks of 128 in the full sequence
P = 128
NCC = EMB // P    # contraction chunks for projections

_cache = {}
LAST_RESULT = None


def _build():
    import concourse.tile as tile
    import concourse.mybir as mybir
    from concourse import bacc
    from concourse.masks import make_identity

    bf16 = mybir.dt.bfloat16
    f32 = mybir.dt.float32
    Exp = mybir.ActivationFunctionType.Exp

    nc = bacc.Bacc(None)
    NS = T // QT
    # xkt is host-staged per 512-key span in exactly the SBUF tile layout
    # [span][c_part 128][c_chunk 8][q 512] so each span is one linear DMA
    xkt = nc.declare_dram_parameter("xkt", [NS, P, NCC, QT], bf16, isOutput=False)
    # weights and masks are host-staged in SBUF layout -> linear DMAs
    wq = nc.declare_dram_parameter("wq", [P, NCC, HEAD], bf16, isOutput=False)
    wkv = nc.declare_dram_parameter("wkv", [P, NCC, 2 * HEAD], bf16, isOutput=False)
    masks = nc.declare_dram_parameter("masks", [P, 8, QT], bf16, isOutput=False)
    out = nc.declare_dram_parameter("out", [NT * QT, HEAD], f32, isOutput=True)

    with tile.TileContext(nc) as tc:
        with (
            tc.tile_pool(name="const", bufs=1) as const,
            tc.tile_pool(name="xk", bufs=4) as xkp,
            tc.tile_pool(name="persist", bufs=1) as persist,
            tc.tile_pool(name="vt", bufs=3) as vtp,
            tc.tile_pool(name="es", bufs=4) as esp,
            tc.tile_pool(name="ot", bufs=2) as otp,
            tc.tile_pool(name="ob", bufs=4) as obp,
            tc.tile_pool(name="small", bufs=4) as smallp,
        ):
            # ---- constants / persistent SBUF ----
            wq_sb = const.tile([P, NCC, HEAD], bf16)
            nc.sync.dma_start(wq_sb[:], wq[:])
            wkv_sb = const.tile([P, NCC, 2 * HEAD], bf16)
            nc.sync.dma_start(wkv_sb[:], wkv[:])
            # masks are not needed until the attention phase; park them on the
            # slow software queue, off the critical first-span DMA path
            mask_sb = const.tile([P, 8, QT], bf16)
            nc.gpsimd.dma_start(mask_sb[:], masks[:])
            id_bf = const.tile([HEAD, HEAD], bf16)
            make_identity(nc, id_bf[:])
            id_f32 = const.tile([HEAD + 1, HEAD + 1], f32)
            make_identity(nc, id_f32[:])

            # kT/qT live duplicated in both partition halves so scores matmuls
            # can alternate PE row-groups (even kc -> rows 0-63, odd kc ->
            # rows 64-127), letting LDWEIGHTS overlap the neighboring matmul
            kt_sb = persist.tile([P, T], bf16, tag="kt")
            qt_sb = persist.tile([P, NT * QT], bf16, tag="qt")
            vaug_sb = persist.tile([P, NKC, HEAD + 1], bf16, tag="vaug")
            nc.vector.memset(vaug_sb[:, :, HEAD], 1.0)

            # ---- interleaved schedule: attention tile i is emitted as soon
            # as its inputs (spans 0..2i+1, their transposes, qT(i)) exist, so
            # attention work fills the DMA-paced projection stretch and
            # projection matmuls fill the exp-paced attention stretch ----
            GRP = 2  # key chunks per exp ACTIVATE (amortizes ACT fixed cost)
            with (
                tc.tile_pool(name="ps_proj", bufs=2, space="PSUM") as ps_proj,
                tc.tile_pool(name="ps_sc", bufs=2, space="PSUM") as ps_sc,
                tc.tile_pool(name="ps_acc", bufs=1, space="PSUM") as ps_acc,
                tc.tile_pool(name="ps_fin", bufs=1, space="PSUM") as ps_fin,
            ):
                ps_kv = ps_q = ps_tr = ps_proj
                vt_tiles = [None] * NS

                def emit_span(s):
                    xs = xkp.tile([P, NCC, QT], bf16, tag="xk")
                    if s == 0:
                        # split the critical first span across both HW queues
                        nc.sync.dma_start(xs[:, 0:NCC // 2, :], xkt[s, :, 0:NCC // 2, :])
                        nc.scalar.dma_start(xs[:, NCC // 2:NCC, :], xkt[s, :, NCC // 2:NCC, :])
                    else:
                        dma_eng = nc.sync if s % 2 == 0 else nc.scalar
                        dma_eng.dma_start(xs[:], xkt[s])
                    ps = ps_kv.tile([P, QT], mybir.dt.float32, tag="p")
                    for j in range(NCC):
                        nc.tensor.matmul(
                            ps[:], lhsT=wkv_sb[:, j, :], rhs=xs[:, j, :],
                            start=(j == 0), stop=(j == NCC - 1),
                        )
                    sl = slice(s * QT, (s + 1) * QT)
                    nc.scalar.copy(kt_sb[0:HEAD, sl], ps[0:HEAD, :])
                    nc.vector.tensor_copy(kt_sb[HEAD:P, sl], ps[0:HEAD, :])
                    vt = vtp.tile([HEAD, QT], bf16, tag="vt")
                    nc.vector.tensor_copy(vt[:], ps[HEAD:P, :])
                    vt_tiles[s] = vt
                    if s % 2 == 0:
                        i = s // 2
                        psq = ps_q.tile([HEAD, QT], mybir.dt.float32, tag="p")
                        for j in range(NCC):
                            nc.tensor.matmul(
                                psq[:], lhsT=wq_sb[:, j, :], rhs=xs[:, j, :],
                                start=(j == 0), stop=(j == NCC - 1),
                            )
                        qsl = slice(i * QT, (i + 1) * QT)
                        nc.scalar.copy(qt_sb[0:HEAD, qsl], psq[:])
                        nc.vector.tensor_copy(qt_sb[HEAD:P, qsl], psq[:])

                def emit_transposes(s):
                    vt = vt_tiles[s]
                    for bb in range(QT // P):
                        tp = ps_tr.tile([P, HEAD], bf16, tag="p")
                        nc.tensor.transpose(tp[:], vt[:, bb * P:(bb + 1) * P], id_bf[:])
                        kc = s * (QT // P) + bb
                        nc.vector.tensor_copy(vaug_sb[:, kc, 0:HEAD], tp[:])

                def emit_attn(i):
                    nk = 8 * i + 8
                    acc = ps_acc.tile([HEAD + 1, QT], mybir.dt.float32)

                    def emit_scores(kc0, g):
                        sc = ps_sc.tile([P, GRP, QT], mybir.dt.float32, tag="sc")
                        for d in range(g):
                            kc = kc0 + d
                            h0 = (kc % 2) * HEAD  # alternate PE row-groups
                            nc.tensor.matmul(
                                sc[:, d, :],
                                lhsT=kt_sb[h0:h0 + HEAD, kc * P:(kc + 1) * P],
                                rhs=qt_sb[h0:h0 + HEAD, i * QT:(i + 1) * QT],
                                start=True, stop=True,
                            )
                        es = esp.tile([P, GRP, QT], bf16, tag="es")
                        nc.scalar.activation(
                            es[:, 0:g, :], sc[:, 0:g, :], Exp, scale=0.125)
                        for d in range(g):
                            kc = kc0 + d
                            if kc >= 8 * i:
                                nc.vector.tensor_mul(
                                    es[:, d, :], es[:, d, :],
                                    mask_sb[:, kc - 8 * i, :],
                                )
                        return es

                    def emit_pv(kc0, g, es):
                        for d in range(g):
                            kc = kc0 + d
                            nc.tensor.matmul(
                                acc[:],
                                lhsT=vaug_sb[:, kc, :],
                                rhs=es[:, d, :],
                                start=(kc == 0), stop=(kc == nk - 1),
                            )

                    from collections import deque
                    pending = deque()
                    kc0 = 0
                    while kc0 < nk:
                        g = min(GRP, nk - kc0)
                        es = emit_scores(kc0, g)
                        pending.append((kc0, g, es))
                        if len(pending) > 2:
                            emit_pv(*pending.popleft())
                        kc0 += g
                    while pending:
                        emit_pv(*pending.popleft())

                    ot = otp.tile([HEAD + 1, QT], mybir.dt.float32, tag="ot")
                    nc.vector.tensor_copy(ot[:], acc[:])
                    for sb in range(QT // P):
                        fp = ps_fin.tile([P, HEAD + 1], mybir.dt.float32)
                        nc.tensor.transpose(
                            fp[:], ot[:, sb * P:(sb + 1) * P], id_f32[:]
                        )
                        rc = smallp.tile([P, 1], mybir.dt.float32, tag="rc")
                        nc.vector.reciprocal(rc[:], fp[:, HEAD:HEAD + 1])
                        ob = obp.tile([P, HEAD], mybir.dt.float32, tag="ob")
                        nc.vector.tensor_scalar_mul(ob[:], fp[:, 0:HEAD], rc[:])
                        r0 = i * QT + sb * P
                        nc.sync.dma_start(out[r0:r0 + P, :], ob[:])

                # interleaved emission: attention tile i right after span
                # 2i+1's transposes; later spans keep streaming behind it
                emit_span(0)
                emit_span(1)
                emit_transposes(0)
                emit_span(2)
                emit_transposes(1)
                emit_attn(0)
                emit_span(3)
                emit_transposes(2)
                emit_span(4)
                emit_transposes(3)
                emit_attn(1)
                emit_span(5)
                emit_transposes(4)
                emit_span(6)
                emit_transposes(5)
                emit_attn(2)
                emit_span(7)
                emit_transposes(6)
                emit_transposes(7)
                emit_attn(3)
    nc.finalize()
    return nc


def _stage_inputs(x, Wq, Wk, Wv):
    bf = ml_dtypes.bfloat16

    def _w_stage(w):  # [1024, h] -> [128, 8, h] matching SBUF tiles
        w = np.asarray(w, dtype=np.float32).astype(bf)
        return np.ascontiguousarray(w.reshape(NCC, P, w.shape[1]).transpose(1, 0, 2))

    wq = _w_stage(Wq)
    wkv = _w_stage(np.concatenate([np.asarray(Wk), np.asarray(Wv)], axis=1))

    # causal mask slabs for the 4 diagonal key chunks of the own tile
    kk = np.arange(P)[:, None]
    qq = np.arange(QT)[None, :]
    tri = [(qq >= (P * j + kk)).astype(bf) for j in range(4)]
    zeros = np.zeros((P, QT), dtype=bf)
    ones = np.ones((P, QT), dtype=bf)
    # device layout [128, 8, 512]
    mask_even = np.ascontiguousarray(
        np.stack(tri + [zeros] * 4).transpose(1, 0, 2))  # own first: future -> 0
    mask_odd = np.ascontiguousarray(
        np.stack(tri + [ones] * 4).transpose(1, 0, 2))   # own second: past -> 1

    in_maps = []
    for b in range(B):
        xbt = np.ascontiguousarray(x[b].T, dtype=np.float32).astype(bf)
        for p in range(2):
            cols = []
            for i in range(NT):
                own = 2 * i + p
                oth = 2 * i + 1 - p
                cols.append(xbt[:, own * QT:(own + 1) * QT])
                cols.append(xbt[:, oth * QT:(oth + 1) * QT])
            staged = np.concatenate(cols, axis=1)  # [1024, 4096]
            # device layout: [span][c_part 128][c_chunk 8][q 512]
            staged = np.ascontiguousarray(
                staged.reshape(NCC, P, T // QT, QT).transpose(2, 1, 0, 3)
            )
            in_maps.append({
                "xkt": staged,
                "wq": wq,
                "wkv": wkv,
                "masks": mask_even if p == 0 else mask_odd,
            })
    return in_maps


def kernel(x, Wq, Wk, Wv):
    global LAST_RESULT
    from concourse.bass_utils import run_bass_kernel_spmd

    x = np.asarray(x)
    if "nc" not in _cache:
        _cache["nc"] = _build()
    nc = _cache["nc"]

    in_maps = _stage_inputs(x, Wq, Wk, Wv)
    trace = bool(int(os.environ.get("ATTN_TRACE", "0")))
    res = run_bass_kernel_spmd(nc, in_maps, core_ids=list(range(8)), trace=trace)
    LAST_RESULT = res

    out = np.empty((B, T, HEAD), dtype=np.float32)
    for b in range(B):
        for p in range(2):
            o = res.results[2 * b + p]["out"]
            for i in range(NT):
                a0 = (2 * i + p) * QT
                out[b, a0:a0 + QT] = o[i * QT:(i + 1) * QT]
    return out


# revision 41
# speedup vs baseline: 1.0879x; 1.0032x over previous
"""Single-head causal attention on 8 TRN2 NeuronCores.

Problem: x:[4,4096,1024] f32, Wq/Wk/Wv:[1024,64] f32.
  q,k,v = x@W*; scores = q@k.T/8 (causal); out = softmax(scores)@v.

Sharding: 2 cores per batch element (B=4 x 2 = 8 cores). Within a batch the
8 query tiles of 512 rows are dealt pairwise: core parity p owns absolute
tiles {2i+p}. Both cores run ONE SPMD program; the causal asymmetry between
even/odd tiles is pushed into per-core *data*: the key columns are staged in
per-core order [own_tile_i, other_tile_i]*4 and the causal masks are inputs.

Device algorithm (per core, all bf16 matmul operands, f32 PSUM accum):
  kvT[:,t]  = [Wk|Wv].T @ xT[:,t]          (stacked projection, kT + vT)
  qT        = Wq.T @ xT[:, own tiles]
  v_aug[kc] = transpose(vT chunk) ++ ones col   (PE transpose)
  scoresT   = kT_chunk.T @ qT_tile          ([128k x 512q] in PSUM)
  esT       = exp(0.125 * scoresT)          (ScalarE; no max-subtract needed:
                                             |scores| < ~4 so exp is safe)
  esT      *= mask (diagonal/padded chunks only)
  outT     += v_aug[kc].T @ esT             ([65 x 512]: 64 numerator rows +
                                             row 64 = softmax denominator)
  out_tile  = transpose(outT) ; out[:,0:64] * (1/out[:,64])
"""

import os
import numpy as np
import ml_dtypes

HEAD = 64
EMB = 1024
B = 4
T = 4096
QT = 512          # queries per logical tile (matmul moving dim)
NT = 4            # logical q tiles per core  (NT*QT = 2048 queries/core)
NKC = 32          # key chunks of 128 in the full sequence
P = 128
NCC = EMB // P    # contraction chunks for projections

_cache = {}
LAST_RESULT = None


def _build():
    import concourse.tile as tile
    import concourse.mybir as mybir
    from concourse import bacc
    from concourse.masks import make_identity

    bf16 = mybir.dt.bfloat16
    f32 = mybir.dt.float32
    Exp = mybir.ActivationFunctionType.Exp

    nc = bacc.Bacc(None)
    NS = T // QT
    # xkt is host-staged per 512-key span in exactly the SBUF tile layout
    # [span][c_part 128][c_chunk 8][q 512] so each span is one linear DMA
    xkt = nc.declare_dram_parameter("xkt", [NS, P, NCC, QT], bf16, isOutput=False)
    # weights and masks are host-staged in SBUF layout -> linear DMAs
    wq = nc.declare_dram_parameter("wq", [P, NCC, HEAD], bf16, isOutput=False)
    wkv = nc.declare_dram_parameter("wkv", [P, NCC, 2 * HEAD], bf16, isOutput=False)
    masks = nc.declare_dram_parameter("masks", [P, 8, QT], bf16, isOutput=False)
    out = nc.declare_dram_parameter("out", [NT * QT, HEAD], f32, isOutput=True)

    with tile.TileContext(nc) as tc:
        with (
            tc.tile_pool(name="const", bufs=1) as const,
            tc.tile_pool(name="xk", bufs=4) as xkp,
            tc.tile_pool(name="persist", bufs=1) as persist,
            tc.tile_pool(name="vt", bufs=3) as vtp,
            tc.tile_pool(name="es", bufs=4) as esp,
            tc.tile_pool(name="ot", bufs=2) as otp,
            tc.tile_pool(name="ob", bufs=4) as obp,
            tc.tile_pool(name="small", bufs=4) as smallp,
        ):
            # ---- constants / persistent SBUF ----
            wq_sb = const.tile([P, NCC, HEAD], bf16)
            nc.scalar.dma_start(wq_sb[:], wq[:])
            wkv_sb = const.tile([P, NCC, 2 * HEAD], bf16)
            nc.scalar.dma_start(wkv_sb[:], wkv[:])
            # masks are not needed until the attention phase; park them on the
            # slow software queue, off the critical first-span DMA path
            mask_sb = const.tile([P, 8, QT], bf16)
            nc.gpsimd.dma_start(mask_sb[:], masks[:])
            id_bf = const.tile([HEAD, HEAD], bf16)
            make_identity(nc, id_bf[:])
            id_f32 = const.tile([HEAD + 1, HEAD + 1], f32)
            make_identity(nc, id_f32[:])

            # kT/qT live duplicated in both partition halves so scores matmuls
            # can alternate PE row-groups (even kc -> rows 0-63, odd kc ->
            # rows 64-127), letting LDWEIGHTS overlap the neighboring matmul
            kt_sb = persist.tile([P, T], bf16, tag="kt")
            qt_sb = persist.tile([P, NT * QT], bf16, tag="qt")
            vaug_sb = persist.tile([P, NKC, HEAD + 1], bf16, tag="vaug")
            nc.vector.memset(vaug_sb[:, :, HEAD], 1.0)

            # ---- interleaved schedule: attention tile i is emitted as soon
            # as its inputs (spans 0..2i+1, their transposes, qT(i)) exist, so
            # attention work fills the DMA-paced projection stretch and
            # projection matmuls fill the exp-paced attention stretch ----
            GRP = 2  # key chunks per exp ACTIVATE (amortizes ACT fixed cost)
            with (
                tc.tile_pool(name="ps_proj", bufs=2, space="PSUM") as ps_proj,
                tc.tile_pool(name="ps_sc", bufs=2, space="PSUM") as ps_sc,
                tc.tile_pool(name="ps_acc", bufs=1, space="PSUM") as ps_acc,
                tc.tile_pool(name="ps_fin", bufs=1, space="PSUM") as ps_fin,
            ):
                ps_kv = ps_q = ps_tr = ps_proj
                vt_tiles = [None] * NS

                def emit_span(s):
                    xs = xkp.tile([P, NCC, QT], bf16, tag="xk")
                    if s == 0:
                        # split the critical first span across both HW queues
                        nc.sync.dma_start(xs[:, 0:NCC // 2, :], xkt[s, :, 0:NCC // 2, :])
                        nc.scalar.dma_start(xs[:, NCC // 2:NCC, :], xkt[s, :, NCC // 2:NCC, :])
                    else:
                        dma_eng = nc.scalar if s % 2 == 0 else nc.sync
                        dma_eng.dma_start(xs[:], xkt[s])
                    ps = ps_kv.tile([P, QT], mybir.dt.float32, tag="p")
                    for j in range(NCC):
                        nc.tensor.matmul(
                            ps[:], lhsT=wkv_sb[:, j, :], rhs=xs[:, j, :],
                            start=(j == 0), stop=(j == NCC - 1),
                        )
                    sl = slice(s * QT, (s + 1) * QT)
                    nc.scalar.copy(kt_sb[0:HEAD, sl], ps[0:HEAD, :])
                    nc.vector.tensor_copy(kt_sb[HEAD:P, sl], ps[0:HEAD, :])
                    vt = vtp.tile([HEAD, QT], bf16, tag="vt")
                    nc.vector.tensor_copy(vt[:], ps[HEAD:P, :])
                    vt_tiles[s] = vt
                    if s % 2 == 0:
                        i = s // 2
                        psq = ps_q.tile([HEAD, QT], mybir.dt.float32, tag="p")
                        for j in range(NCC):
                            nc.tensor.matmul(
                                psq[:], lhsT=wq_sb[:, j, :], rhs=xs[:, j, :],
                                start=(j == 0), stop=(j == NCC - 1),
                            )
                        qsl = slice(i * QT, (i + 1) * QT)
                        nc.scalar.copy(qt_sb[0:HEAD, qsl], psq[:])
                        nc.vector.tensor_copy(qt_sb[HEAD:P, qsl], psq[:])

                def emit_transposes(s):
                    vt = vt_tiles[s]
                    for bb in range(QT // P):
                        tp = ps_tr.tile([P, HEAD], bf16, tag="p")
                        nc.tensor.transpose(tp[:], vt[:, bb * P:(bb + 1) * P], id_bf[:])
                        kc = s * (QT // P) + bb
                        nc.vector.tensor_copy(vaug_sb[:, kc, 0:HEAD], tp[:])

                def emit_attn(i):
                    nk = 8 * i + 8
                    acc = ps_acc.tile([HEAD + 1, QT], mybir.dt.float32)

                    def emit_scores(kc0, g):
                        sc = ps_sc.tile([P, GRP, QT], mybir.dt.float32, tag="sc")
                        for d in range(g):
                            kc = kc0 + d
                            h0 = (kc % 2) * HEAD  # alternate PE row-groups
                            nc.tensor.matmul(
                                sc[:, d, :],
                                lhsT=kt_sb[h0:h0 + HEAD, kc * P:(kc + 1) * P],
                                rhs=qt_sb[h0:h0 + HEAD, i * QT:(i + 1) * QT],
                                start=True, stop=True,
                            )
                        es = esp.tile([P, GRP, QT], bf16, tag="es")
                        nc.scalar.activation(
                            es[:, 0:g, :], sc[:, 0:g, :], Exp, scale=0.125)
                        for d in range(g):
                            kc = kc0 + d
                            if kc >= 8 * i:
                                nc.vector.tensor_mul(
                                    es[:, d, :], es[:, d, :],
                                    mask_sb[:, kc - 8 * i, :],
                                )
                        return es

                    def emit_pv(kc0, g, es):
                        for d in range(g):
                            kc = kc0 + d
                            nc.tensor.matmul(
                                acc[:],
                                lhsT=vaug_sb[:, kc, :],
                                rhs=es[:, d, :],
                                start=(kc == 0), stop=(kc == nk - 1),
                            )

                    from collections import deque
                    pending = deque()
                    kc0 = 0
                    while kc0 < nk:
                        g = min(GRP, nk - kc0)
                        es = emit_scores(kc0, g)
                        pending.append((kc0, g, es))
                        if len(pending) > 2:
                            emit_pv(*pending.popleft())
                        kc0 += g
                    while pending:
                        emit_pv(*pending.popleft())

                    ot = otp.tile([HEAD + 1, QT], mybir.dt.float32, tag="ot")
                    nc.vector.tensor_copy(ot[:], acc[:])
                    for sb in range(QT // P):
                        fp = ps_fin.tile([P, HEAD + 1], mybir.dt.float32)
                        nc.tensor.transpose(
                            fp[:], ot[:, sb * P:(sb + 1) * P], id_f32[:]
                        )
                        rc = smallp.tile([P, 1], mybir.dt.float32, tag="rc")
                        nc.vector.reciprocal(rc[:], fp[:, HEAD:HEAD + 1])
                        ob = obp.tile([P, HEAD], mybir.dt.float32, tag="ob")
                        nc.vector.tensor_scalar_mul(ob[:], fp[:, 0:HEAD], rc[:])
                        r0 = i * QT + sb * P
                        nc.sync.dma_start(out[r0:r0 + P, :], ob[:])

                # interleaved emission: attention tile i right after span
                # 2i+1's transposes; later spans keep streaming behind it
                emit_span(0)
                emit_span(1)
                emit_transposes(0)
                emit_span(2)
                emit_transposes(1)
                emit_attn(0)
                emit_span(3)
                emit_transposes(2)
                emit_span(4)
                emit_transposes(3)
                emit_attn(1)
                emit_span(5)
                emit_transposes(4)
                emit_span(6)
                emit_transposes(5)
                emit_attn(2)
                emit_span(7)
                emit_transposes(6)
                emit_transposes(7)
                emit_attn(3)
    nc.finalize()
    return nc


def _stage_inputs(x, Wq, Wk, Wv):
    bf = ml_dtypes.bfloat16

    def _w_stage(w):  # [1024, h] -> [128, 8, h] matching SBUF tiles
        w = np.asarray(w, dtype=np.float32).astype(bf)
        return np.ascontiguousarray(w.reshape(NCC, P, w.shape[1]).transpose(1, 0, 2))

    wq = _w_stage(Wq)
    wkv = _w_stage(np.concatenate([np.asarray(Wk), np.asarray(Wv)], axis=1))

    # causal mask slabs for the 4 diagonal key chunks of the own tile
    kk = np.arange(P)[:, None]
    qq = np.arange(QT)[None, :]
    tri = [(qq >= (P * j + kk)).astype(bf) for j in range(4)]
    zeros = np.zeros((P, QT), dtype=bf)
    ones = np.ones((P, QT), dtype=bf)
    # device layout [128, 8, 512]
    mask_even = np.ascontiguousarray(
        np.stack(tri + [zeros] * 4).transpose(1, 0, 2))  # own first: future -> 0
    mask_odd = np.ascontiguousarray(
        np.stack(tri + [ones] * 4).transpose(1, 0, 2))   # own second: past -> 1

    in_maps = []
    for b in range(B):
        xbt = np.ascontiguousarray(x[b].T, dtype=np.float32).astype(bf)
        for p in range(2):
            cols = []
            for i in range(NT):
                own = 2 * i + p
                oth = 2 * i + 1 - p
                cols.append(xbt[:, own * QT:(own + 1) * QT])
                cols.append(xbt[:, oth * QT:(oth + 1) * QT])
            staged = np.concatenate(cols, axis=1)  # [1024, 4096]
            # device layout: [span][c_part 128][c_chunk 8][q 512]
            staged = np.ascontiguousarray(
                staged.reshape(NCC, P, T // QT, QT).transpose(2, 1, 0, 3)
            )
            in_maps.append({
                "xkt": staged,
                "wq": wq,
                "wkv": wkv,
                "masks": mask_even if p == 0 else mask_odd,
            })
    return in_maps


def kernel(x, Wq, Wk, Wv):
    global LAST_RESULT
    from concourse.bass_utils import run_bass_kernel_spmd

    x = np.asarray(x)
    if "nc" not in _cache:
        _cache["nc"] = _build()
    nc = _cache["nc"]

    in_maps = _stage_inputs(x, Wq, Wk, Wv)
    trace = bool(int(os.environ.get("ATTN_TRACE", "0")))
    res = run_bass_kernel_spmd(nc, in_maps, core_ids=list(range(8)), trace=trace)
    LAST_RESULT = res

    out = np.empty((B, T, HEAD), dtype=np.float32)
    for b in range(B):
        for p in range(2):
            o = res.results[2 * b + p]["out"]
            for i in range(NT):
                a0 = (2 * i + p) * QT
                out[b, a0:a0 + QT] = o[i * QT:(i + 1) * QT]
    return out


# revision 42
# speedup vs baseline: 1.1216x; 1.0310x over previous
"""Single-head causal attention on 8 TRN2 NeuronCores.

Problem: x:[4,4096,1024] f32, Wq/Wk/Wv:[1024,64] f32.
  q,k,v = x@W*; scores = q@k.T/8 (causal); out = softmax(scores)@v.

Sharding: 2 cores per batch element (B=4 x 2 = 8 cores). Within a batch the
8 query tiles of 512 rows are dealt pairwise: core parity p owns absolute
tiles {2i+p}. Both cores run ONE SPMD program; the causal asymmetry between
even/odd tiles is pushed into per-core *data*: the key columns are staged in
per-core order [own_tile_i, other_tile_i]*4 and the causal masks are inputs.

Device algorithm (per core, all bf16 matmul operands, f32 PSUM accum):
  kvT[:,t]  = [Wk|Wv].T @ xT[:,t]          (stacked projection, kT + vT)
  qT        = Wq.T @ xT[:, own tiles]
  v_aug[kc] = transpose(vT chunk) ++ ones col   (PE transpose)
  scoresT   = kT_chunk.T @ qT_tile          ([128k x 512q] in PSUM)
  esT       = exp(0.125 * scoresT)          (ScalarE; no max-subtract needed:
                                             |scores| < ~4 so exp is safe)
  esT      *= mask (diagonal/padded chunks only)
  outT     += v_aug[kc].T @ esT             ([65 x 512]: 64 numerator rows +
                                             row 64 = softmax denominator)
  out_tile  = transpose(outT) ; out[:,0:64] * (1/out[:,64])
"""

import os
import numpy as np
import ml_dtypes

HEAD = 64
EMB = 1024
B = 4
T = 4096
QT = 512          # queries per logical tile (matmul moving dim)
NT = 4            # logical q tiles per core  (NT*QT = 2048 queries/core)
NKC = 32          # key chunks of 128 in the full sequence
P = 128
NCC = EMB // P    # contraction chunks for projections

_cache = {}
LAST_RESULT = None


def _build():
    import concourse.tile as tile
    import concourse.mybir as mybir
    from concourse import bacc
    from concourse.masks import make_identity

    bf16 = mybir.dt.bfloat16
    f32 = mybir.dt.float32
    Exp = mybir.ActivationFunctionType.Exp

    nc = bacc.Bacc(None)
    NS = T // QT
    # xkt is host-staged per 512-key span in exactly the SBUF tile layout
    # [span][c_part 128][c_chunk 8][q 512] so each span is one linear DMA
    xkt = nc.declare_dram_parameter("xkt", [NS, P, NCC, QT], bf16, isOutput=False)
    # weights and masks are host-staged in SBUF layout -> linear DMAs
    wq = nc.declare_dram_parameter("wq", [P, NCC, HEAD], bf16, isOutput=False)
    wkv = nc.declare_dram_parameter("wkv", [P, NCC, 2 * HEAD], bf16, isOutput=False)
    masks = nc.declare_dram_parameter("masks", [P, 8, QT], bf16, isOutput=False)
    out = nc.declare_dram_parameter("out", [NT * QT, HEAD], f32, isOutput=True)

    with tile.TileContext(nc) as tc:
        with (
            tc.tile_pool(name="const", bufs=1) as const,
            tc.tile_pool(name="xk", bufs=4) as xkp,
            tc.tile_pool(name="persist", bufs=1) as persist,
            tc.tile_pool(name="vt", bufs=3) as vtp,
            tc.tile_pool(name="es", bufs=4) as esp,
            tc.tile_pool(name="ot", bufs=2) as otp,
            tc.tile_pool(name="ob", bufs=4) as obp,
            tc.tile_pool(name="small", bufs=4) as smallp,
        ):
            # ---- constants / persistent SBUF ----
            wq_sb = const.tile([P, NCC, HEAD], bf16)
            nc.scalar.dma_start(wq_sb[:], wq[:])
            wkv_sb = const.tile([P, NCC, 2 * HEAD], bf16)
            nc.scalar.dma_start(wkv_sb[:], wkv[:])
            id_bf = const.tile([HEAD, HEAD], bf16)
            make_identity(nc, id_bf[:])
            id_f32 = const.tile([HEAD + 1, HEAD + 1], f32)
            make_identity(nc, id_f32[:])
            # masks are not needed until the attention phase; park them on the
            # slow software queue (issued after the identities gpsimd builds),
            # off the critical first-span DMA path
            mask_sb = const.tile([P, 8, QT], bf16)
            nc.gpsimd.dma_start(mask_sb[:], masks[:])

            # kT/qT live duplicated in both partition halves so scores matmuls
            # can alternate PE row-groups (even kc -> rows 0-63, odd kc ->
            # rows 64-127), letting LDWEIGHTS overlap the neighboring matmul
            kt_sb = persist.tile([P, T], bf16, tag="kt")
            qt_sb = persist.tile([P, NT * QT], bf16, tag="qt")
            vaug_sb = persist.tile([P, NKC, HEAD + 1], bf16, tag="vaug")
            nc.vector.memset(vaug_sb[:, :, HEAD], 1.0)

            # ---- interleaved schedule: attention tile i is emitted as soon
            # as its inputs (spans 0..2i+1, their transposes, qT(i)) exist, so
            # attention work fills the DMA-paced projection stretch and
            # projection matmuls fill the exp-paced attention stretch ----
            GRP = 2  # key chunks per exp ACTIVATE (amortizes ACT fixed cost)
            with (
                tc.tile_pool(name="ps_proj", bufs=2, space="PSUM") as ps_proj,
                tc.tile_pool(name="ps_sc", bufs=2, space="PSUM") as ps_sc,
                tc.tile_pool(name="ps_acc", bufs=1, space="PSUM") as ps_acc,
                tc.tile_pool(name="ps_fin", bufs=1, space="PSUM") as ps_fin,
            ):
                ps_kv = ps_q = ps_tr = ps_proj
                vt_tiles = [None] * NS

                def emit_span(s):
                    xs = xkp.tile([P, NCC, QT], bf16, tag="xk")
                    if s == 0:
                        # split the critical first span across both HW queues
                        nc.sync.dma_start(xs[:, 0:NCC // 2, :], xkt[s, :, 0:NCC // 2, :])
                        nc.scalar.dma_start(xs[:, NCC // 2:NCC, :], xkt[s, :, NCC // 2:NCC, :])
                    else:
                        dma_eng = nc.scalar if s % 2 == 0 else nc.sync
                        dma_eng.dma_start(xs[:], xkt[s])
                    ps = ps_kv.tile([P, QT], mybir.dt.float32, tag="p")
                    for j in range(NCC):
                        nc.tensor.matmul(
                            ps[:], lhsT=wkv_sb[:, j, :], rhs=xs[:, j, :],
                            start=(j == 0), stop=(j == NCC - 1),
                        )
                    sl = slice(s * QT, (s + 1) * QT)
                    nc.scalar.copy(kt_sb[0:HEAD, sl], ps[0:HEAD, :])
                    nc.vector.tensor_copy(kt_sb[HEAD:P, sl], ps[0:HEAD, :])
                    vt = vtp.tile([HEAD, QT], bf16, tag="vt")
                    nc.vector.tensor_copy(vt[:], ps[HEAD:P, :])
                    vt_tiles[s] = vt
                    if s % 2 == 0:
                        i = s // 2
                        psq = ps_q.tile([HEAD, QT], mybir.dt.float32, tag="p")
                        for j in range(NCC):
                            nc.tensor.matmul(
                                psq[:], lhsT=wq_sb[:, j, :], rhs=xs[:, j, :],
                                start=(j == 0), stop=(j == NCC - 1),
                            )
                        qsl = slice(i * QT, (i + 1) * QT)
                        nc.scalar.copy(qt_sb[0:HEAD, qsl], psq[:])
                        nc.vector.tensor_copy(qt_sb[HEAD:P, qsl], psq[:])

                def emit_transposes(s):
                    vt = vt_tiles[s]
                    for bb in range(QT // P):
                        tp = ps_tr.tile([P, HEAD], bf16, tag="p")
                        nc.tensor.transpose(tp[:], vt[:, bb * P:(bb + 1) * P], id_bf[:])
                        kc = s * (QT // P) + bb
                        nc.vector.tensor_copy(vaug_sb[:, kc, 0:HEAD], tp[:])

                def emit_attn(i):
                    nk = 8 * i + 8
                    acc = ps_acc.tile([HEAD + 1, QT], mybir.dt.float32)

                    def emit_scores(kc0, g):
                        sc = ps_sc.tile([P, GRP, QT], mybir.dt.float32, tag="sc")
                        for d in range(g):
                            kc = kc0 + d
                            h0 = (kc % 2) * HEAD  # alternate PE row-groups
                            nc.tensor.matmul(
                                sc[:, d, :],
                                lhsT=kt_sb[h0:h0 + HEAD, kc * P:(kc + 1) * P],
                                rhs=qt_sb[h0:h0 + HEAD, i * QT:(i + 1) * QT],
                                start=True, stop=True,
                            )
                        es = esp.tile([P, GRP, QT], bf16, tag="es")
                        nc.scalar.activation(
                            es[:, 0:g, :], sc[:, 0:g, :], Exp, scale=0.125)
                        for d in range(g):
                            kc = kc0 + d
                            if kc >= 8 * i:
                                nc.vector.tensor_mul(
                                    es[:, d, :], es[:, d, :],
                                    mask_sb[:, kc - 8 * i, :],
                                )
                        return es

                    def emit_pv(kc0, g, es):
                        for d in range(g):
                            kc = kc0 + d
                            nc.tensor.matmul(
                                acc[:],
                                lhsT=vaug_sb[:, kc, :],
                                rhs=es[:, d, :],
                                start=(kc == 0), stop=(kc == nk - 1),
                            )

                    from collections import deque
                    pending = deque()
                    kc0 = 0
                    while kc0 < nk:
                        g = min(GRP, nk - kc0)
                        es = emit_scores(kc0, g)
                        pending.append((kc0, g, es))
                        if len(pending) > 2:
                            emit_pv(*pending.popleft())
                        kc0 += g
                    while pending:
                        emit_pv(*pending.popleft())

                    ot = otp.tile([HEAD + 1, QT], mybir.dt.float32, tag="ot")
                    nc.vector.tensor_copy(ot[:], acc[:])
                    for sb in range(QT // P):
                        fp = ps_fin.tile([P, HEAD + 1], mybir.dt.float32)
                        nc.tensor.transpose(
                            fp[:], ot[:, sb * P:(sb + 1) * P], id_f32[:]
                        )
                        rc = smallp.tile([P, 1], mybir.dt.float32, tag="rc")
                        nc.vector.reciprocal(rc[:], fp[:, HEAD:HEAD + 1])
                        ob = obp.tile([P, HEAD], mybir.dt.float32, tag="ob")
                        nc.vector.tensor_scalar_mul(ob[:], fp[:, 0:HEAD], rc[:])
                        r0 = i * QT + sb * P
                        nc.sync.dma_start(out[r0:r0 + P, :], ob[:])

                # interleaved emission: attention tile i right after span
                # 2i+1's transposes; later spans keep streaming behind it
                emit_span(0)
                emit_span(1)
                emit_transposes(0)
                emit_span(2)
                emit_transposes(1)
                emit_attn(0)
                emit_span(3)
                emit_transposes(2)
                emit_span(4)
                emit_transposes(3)
                emit_attn(1)
                emit_span(5)
                emit_transposes(4)
                emit_span(6)
                emit_transposes(5)
                emit_attn(2)
                emit_span(7)
                emit_transposes(6)
                emit_transposes(7)
                emit_attn(3)
    nc.finalize()
    return nc


def _stage_inputs(x, Wq, Wk, Wv):
    bf = ml_dtypes.bfloat16

    def _w_stage(w):  # [1024, h] -> [128, 8, h] matching SBUF tiles
        w = np.asarray(w, dtype=np.float32).astype(bf)
        return np.ascontiguousarray(w.reshape(NCC, P, w.shape[1]).transpose(1, 0, 2))

    wq = _w_stage(Wq)
    wkv = _w_stage(np.concatenate([np.asarray(Wk), np.asarray(Wv)], axis=1))

    # causal mask slabs for the 4 diagonal key chunks of the own tile
    kk = np.arange(P)[:, None]
    qq = np.arange(QT)[None, :]
    tri = [(qq >= (P * j + kk)).astype(bf) for j in range(4)]
    zeros = np.zeros((P, QT), dtype=bf)
    ones = np.ones((P, QT), dtype=bf)
    # device layout [128, 8, 512]
    mask_even = np.ascontiguousarray(
        np.stack(tri + [zeros] * 4).transpose(1, 0, 2))  # own first: future -> 0
    mask_odd = np.ascontiguousarray(
        np.stack(tri + [ones] * 4).transpose(1, 0, 2))   # own second: past -> 1

    in_maps = []
    for b in range(B):
        xbt = np.ascontiguousarray(x[b].T, dtype=np.float32).astype(bf)
        for p in range(2):
            cols = []
            for i in range(NT):
                own = 2 * i + p
                oth = 2 * i + 1 - p
                cols.append(xbt[:, own * QT:(own + 1) * QT])
                cols.append(xbt[:, oth * QT:(oth + 1) * QT])
            staged = np.concatenate(cols, axis=1)  # [1024, 4096]
            # device layout: [span][c_part 128][c_chunk 8][q 512]
            staged = np.ascontiguousarray(
                staged.reshape(NCC, P, T // QT, QT).transpose(2, 1, 0, 3)
            )
            in_maps.append({
                "xkt": staged,
                "wq": wq,
                "wkv": wkv,
                "masks": mask_even if p == 0 else mask_odd,
            })
    return in_maps


def kernel(x, Wq, Wk, Wv):
    global LAST_RESULT
    from concourse.bass_utils import run_bass_kernel_spmd

    x = np.asarray(x)
    if "nc" not in _cache:
        _cache["nc"] = _build()
    nc = _cache["nc"]

    in_maps = _stage_inputs(x, Wq, Wk, Wv)
    trace = bool(int(os.environ.get("ATTN_TRACE", "0")))
    res = run_bass_kernel_spmd(nc, in_maps, core_ids=list(range(8)), trace=trace)
    LAST_RESULT = res

    out = np.empty((B, T, HEAD), dtype=np.float32)
    for b in range(B):
        for p in range(2):
            o = res.results[2 * b + p]["out"]
            for i in range(NT):
                a0 = (2 * i + p) * QT
                out[b, a0:a0 + QT] = o[i * QT:(i + 1) * QT]
    return out
